# revision 14
# baseline (speedup 1.0000x reference)
"""Trainium2 Bass kernel for nn_Attention (Bahdanau-style additive attention).

Computation (reference):
    enc = encoder_outputs.transpose(1, 0, 2)            # [B, S, 2H]
    e_proj = enc @ w_e.T                                # [B, S, H]
    energy = tanh(h_proj[:, None, :] + e_proj + b)      # [B, S, H]
    att = energy @ v_w                                  # [B, S]
    out = softmax(att, axis=1)

Strategy: data-parallel over batch, 4 batch rows per core on 8 cores.
Per core, the big matmul (S x 2H) @ (2H x H) runs in bf16 on the PE:
  - encoder slice is DMA-transposed (xbar) from DRAM bf16 [S, 2H] into
    SBUF [128, 16, SG] so the contraction dim (e) lands on partitions;
    all 4 s-groups of a batch row stay resident so each weight chunk
    (the PE stationary operand) is reused for 4 matmuls and LDWEIGHTS
    stays hidden
  - psum[h_chunk(128), s(512)] accumulates over 16 e-chunks
  - ACT applies tanh with the per-partition bias c_b = h_proj + attn_b
    (h_proj is tiny: computed on host in fp32)
  - the v-dot reduction over h runs as an M=1 matmul accumulated over
    8 h-chunks; the 4 s-groups of a batch row share one PSUM bank at
    partitions {0,32,64,96} via tile_position
All DMAs (including xbar transposes) execute serially in emission order
(Tile serializes DMATranspose vs DMACopy transitions), so the weight
load is split by h-slice and only slice 0 gates the first matmuls; the
rest stream in behind batch row 0's transposes.
Softmax over S (tiny, [32, 2048]) runs on host in fp32.
"""

import sys

try:
    import concourse.bass as bass  # noqa: F401
except ImportError:
    sys.path.insert(0, "/opt/trn_rl_repo")

import numpy as np
import ml_dtypes

import concourse.bacc as bacc
import concourse.mybir as mybir
import concourse.tile as tile
from concourse.bass_utils import run_bass_kernel_spmd

HID = 1024
BATCH = 32
SRC_LEN = 2048

N_CORES = 8
B_LOC = BATCH // N_CORES      # 4
E = 2 * HID                   # 2048
SG = 512                      # matmul moving free dim (s per group)
N_SG = SRC_LEN // SG          # 4
N_EC = E // 128               # 16 e-chunks
N_HC = HID // 128             # 8 h-chunks

f32 = mybir.dt.float32
bf16 = mybir.dt.bfloat16

_NC_CACHE = {}


def _build():
    nc = bacc.Bacc(
        "TRN2", target_bir_lowering=False, debug=False, num_devices=N_CORES
    )
    enc = nc.declare_dram_parameter("enc", [B_LOC, SRC_LEN, E], bf16, isOutput=False)
    wT = nc.declare_dram_parameter("wT", [N_HC, 128, N_EC * 128], bf16, isOutput=False)
    cb = nc.declare_dram_parameter("cb", [128, B_LOC * N_HC], f32, isOutput=False)
    vT = nc.declare_dram_parameter("vT", [128, N_HC], bf16, isOutput=False)
    att = nc.declare_dram_parameter("att", [N_SG, B_LOC, SG], f32, isOutput=True)

    with tile.TileContext(nc) as tc:
        with (
            tc.tile_pool(name="const", bufs=1) as const_pool,
            tc.tile_pool(name="encT", bufs=2 * N_SG) as encT_pool,
            tc.tile_pool(name="energy", bufs=8) as en_pool,
            tc.tile_pool(name="attsb", bufs=1) as att_pool,
            tc.tile_pool(name="psum", bufs=7, space="PSUM") as psum_pool,
            tc.tile_pool(name="attps", bufs=1, space="PSUM") as attps_pool,
        ):
            # weights stored h-slice-major: w_sb[:, hc, c, :] is the [128,128]
            # stationary for (e-chunk c, h-chunk hc); the host pre-lays-out wT
            # as [hc][p][c*128+h'] so each h-slice is one fully-contiguous DMA
            w_sb = const_pool.tile([128, N_HC, N_EC, 128], bf16)
            cb_sb = const_pool.tile([128, B_LOC * N_HC], f32)
            vT_sb = const_pool.tile([128, N_HC], bf16)
            att_all = att_pool.tile([128, B_LOC * SG], f32)

            def load_w_slice(hc):
                nc.sync.dma_start(
                    w_sb[:, hc].rearrange("p c h -> p (c h)"),
                    wT[hc],
                )

            # h-slice 0 of the weights + the small consts: everything the
            # first matmul burst needs; slices 1-2 follow the first transpose
            load_w_slice(0)
            nc.sync.dma_start(cb_sb[:], cb[:])
            nc.sync.dma_start(vT_sb[:], vT[:])

            # warmup tanh: pulls the ACT LUT-table-load dependency off the
            # first real tanh so no instruction needs multiple sync waits
            warm = const_pool.tile([128, 1], f32)
            nc.scalar.activation(
                warm[:], cb_sb[:, 0:1], mybir.ActivationFunctionType.Tanh
            )

            pending = []      # (att_ps, energy, hc, sg)
            done_banks = []   # (att_ps, b) fully accumulated

            def emit_vdots():
                while pending:
                    p_att, p_en, p_hc, p_sg = pending.pop(0)
                    nc.tensor.matmul(
                        p_att[32 * p_sg:32 * p_sg + 1, :],
                        lhsT=vT_sb[:, p_hc:p_hc + 1],
                        rhs=p_en[:],
                        start=(p_hc == 0),
                        stop=(p_hc == N_HC - 1),
                        tile_position=(0, 32 * p_sg),
                    )

            def drain_banks():
                while done_banks:
                    d_ps, d_b = done_banks.pop(0)
                    nc.vector.tensor_copy(
                        att_all[:, d_b * SG:(d_b + 1) * SG], d_ps[:]
                    )

            HALF = N_EC // 2

            def transpose_group(b, sg, split):
                encT = encT_pool.tile(
                    [128, N_EC, SG], bf16, tag="encT", name=f"encT_{b}_{sg}"
                )
                n_parts = 2 if split else 1
                step = N_EC // n_parts
                for part in range(n_parts):
                    nc.sync.dma_start(
                        out=encT[:, part * step:(part + 1) * step, :],
                        in_=enc[
                            b,
                            sg * SG:(sg + 1) * SG,
                            part * step * 128:(part + 1) * step * 128,
                        ],
                        transpose=True,
                    )
                return encT

            def tanh_energy(pss, b, hc, att_ps):
                for sg in range(N_SG):
                    energy = en_pool.tile(
                        [128, SG], bf16, tag="en", name=f"en_{b}_{hc}_{sg}"
                    )
                    nc.scalar.activation(
                        energy[:], pss[sg][:],
                        mybir.ActivationFunctionType.Tanh,
                        bias=cb_sb[:, b * N_HC + hc:b * N_HC + hc + 1],
                        scale=1.0,
                    )
                    pending.append((att_ps, energy, hc, sg))

            N_WARM_HC = 3  # h-chunks runnable per tile during the startup ramp

            for b in range(B_LOC):
                if b == 0:
                    # serial DMA chain ordered so the PE is never starved:
                    # sg0's tile, then weight slices 1-2 (one burst each of
                    # ramp work per tile), remaining tiles, remaining slices
                    encTs = [transpose_group(0, 0, split=True)]
                    load_w_slice(1)
                    load_w_slice(2)
                    encTs += [
                        transpose_group(0, sg, split=True) for sg in range(1, N_SG)
                    ]
                    for hc in range(N_WARM_HC, N_HC):
                        load_w_slice(hc)
                else:
                    encTs = [
                        transpose_group(b, sg, split=False) for sg in range(N_SG)
                    ]
                att_ps = attps_pool.tile(
                    [128, SG], f32, tag="attps", name=f"attps_{b}"
                )
                hc_start = 0
                if b == 0:
                    # startup ramp: consume each s-group tile as its transpose
                    # lands, running h-chunks 0-2 per tile (their weight
                    # slices are the only ones loaded yet)
                    hc_start = N_WARM_HC
                    for sg in range(N_SG):
                        for hc in range(N_WARM_HC):
                            ps = psum_pool.tile(
                                [128, SG], f32, tag="ps", name=f"ps0_{hc}_{sg}"
                            )
                            for c in range(N_EC):
                                nc.tensor.matmul(
                                    ps[:],
                                    lhsT=w_sb[:, hc, c, :],
                                    rhs=encTs[sg][:, c, :],
                                    start=(c == 0),
                                    stop=(c == N_EC - 1),
                                )
                                if hc == 0 and c == 1:
                                    emit_vdots()
                            energy = en_pool.tile(
                                [128, SG], bf16, tag="en", name=f"en0_{hc}_{sg}"
                            )
                            nc.scalar.activation(
                                energy[:], ps[:],
                                mybir.ActivationFunctionType.Tanh,
                                bias=cb_sb[:, 0 * N_HC + hc:0 * N_HC + hc + 1],
                                scale=1.0,
                            )
                            pending.append((att_ps, energy, hc, sg))
                for hc in range(hc_start, N_HC):
                    last_burst = b == B_LOC - 1 and hc == N_HC - 1
                    if last_burst:
                        # tail: per-s-group bursts with eagerly interleaved
                        # tanh + v-dot so only sg3's epilogue trails the MMs
                        for sg in range(N_SG):
                            ps = psum_pool.tile(
                                [128, SG], f32, tag="ps", name=f"ps_t_{sg}"
                            )
                            for c in range(N_EC):
                                nc.tensor.matmul(
                                    ps[:],
                                    lhsT=w_sb[:, hc, c, :],
                                    rhs=encTs[sg][:, c, :],
                                    start=(c == 0),
                                    stop=(c == N_EC - 1),
                                )
                                if c == 1:
                                    emit_vdots()
                            energy = en_pool.tile(
                                [128, SG], bf16, tag="en", name=f"en_t_{sg}"
                            )
                            nc.scalar.activation(
                                energy[:], ps[:],
                                mybir.ActivationFunctionType.Tanh,
                                bias=cb_sb[:, b * N_HC + hc:b * N_HC + hc + 1],
                                scale=1.0,
                            )
                            pending.append((att_ps, energy, hc, sg))
                        continue
                    pss = [
                        psum_pool.tile([128, SG], f32, tag="ps", name=f"ps_{b}_{hc}_{i}")
                        for i in range(N_SG)
                    ]
                    for c in range(N_EC):
                        for sg in range(N_SG):
                            nc.tensor.matmul(
                                pss[sg][:],
                                lhsT=w_sb[:, hc, c, :],
                                rhs=encTs[sg][:, c, :],
                                start=(c == 0),
                                stop=(c == N_EC - 1),
                            )
                        if c == 1:
                            emit_vdots()
                            drain_banks()
                    tanh_energy(pss, b, hc, att_ps)
                done_banks.append((att_ps, b))
            emit_vdots()
            drain_banks()
            nc.sync.dma_start(
                att[:],
                att_all.rearrange("(q t) (b s) -> q t b s", t=32, b=B_LOC)[:, 0],
            )
    nc.compile()
    return nc


def _get_nc():
    if "nc" not in _NC_CACHE:
        _NC_CACHE["nc"] = _build()
    return _NC_CACHE["nc"]


def kernel(hidden, encoder_outputs, attn_w, attn_b, v_w, _trace=False):
    hidden = np.asarray(hidden, dtype=np.float32)
    encoder_outputs = np.asarray(encoder_outputs, dtype=np.float32)
    attn_w = np.asarray(attn_w, dtype=np.float32)
    attn_b = np.asarray(attn_b, dtype=np.float32)
    v_w = np.asarray(v_w, dtype=np.float32)

    # host-side prologue (tiny): h_proj + bias
    c_b = hidden @ attn_w[:, :HID].T + attn_b          # [B, H] fp32
    w_e = attn_w[:, HID:]                                  # [H, E]
    wT_bf = np.ascontiguousarray(
        w_e.reshape(N_HC, 128, N_EC, 128).transpose(0, 3, 2, 1)
        .reshape(N_HC, 128, N_EC * 128)
    ).astype(ml_dtypes.bfloat16)
    vT_dev = np.ascontiguousarray(
        v_w.reshape(N_HC, 128).T
    ).astype(ml_dtypes.bfloat16)

    nc = _get_nc()
    in_maps = []
    for core in range(N_CORES):
        b0 = core * B_LOC
        enc_bf = np.ascontiguousarray(
            encoder_outputs[:, b0:b0 + B_LOC, :].transpose(1, 0, 2)
        ).astype(ml_dtypes.bfloat16)
        cb_dev = np.ascontiguousarray(
            c_b[b0:b0 + B_LOC]
            .reshape(B_LOC, N_HC, 128)
            .transpose(2, 0, 1)
            .reshape(128, B_LOC * N_HC)
        )
        in_maps.append({"enc": enc_bf, "wT": wT_bf, "cb": cb_dev, "vT": vT_dev})

    res = run_bass_kernel_spmd(
        nc, in_maps, core_ids=list(range(N_CORES)), trace=_trace
    )
    if _trace:
        _NC_CACHE["last_result"] = res

    att = np.concatenate(
        [
            res.results[c]["att"].transpose(1, 0, 2).reshape(B_LOC, SRC_LEN)
            for c in range(N_CORES)
        ],
        axis=0,
    )  # [B, S] logits, fp32

    # host softmax over S
    m = att.max(axis=1, keepdims=True)
    e = np.exp(att - m)
    out = e / e.sum(axis=1, keepdims=True)
    return out.astype(np.float32)


# revision 16
# speedup vs baseline: 1.0152x; 1.0152x over previous
"""Trainium2 Bass kernel for nn_Attention (Bahdanau-style additive attention).

Computation (reference):
    enc = encoder_outputs.transpose(1, 0, 2)            # [B, S, 2H]
    e_proj = enc @ w_e.T                                # [B, S, H]
    energy = tanh(h_proj[:, None, :] + e_proj + b)      # [B, S, H]
    att = energy @ v_w                                  # [B, S]
    out = softmax(att, axis=1)

Strategy: data-parallel over batch, 4 batch rows per core on 8 cores.
Per core, the big matmul (S x 2H) @ (2H x H) runs in bf16 on the PE:
  - encoder slice is DMA-transposed (xbar) from DRAM bf16 [S, 2H] into
    SBUF [128, 16, SG] so the contraction dim (e) lands on partitions;
    all 4 s-groups of a batch row stay resident so each weight chunk
    (the PE stationary operand) is reused for 4 matmuls and LDWEIGHTS
    stays hidden
  - psum[h_chunk(128), s(512)] accumulates over 16 e-chunks
  - ACT applies tanh with the per-partition bias c_b = h_proj + attn_b
    (h_proj is tiny: computed on host in fp32)
  - the v-dot reduction over h runs as an M=1 matmul accumulated over
    8 h-chunks; the 4 s-groups of a batch row share one PSUM bank at
    partitions {0,32,64,96} via tile_position
All DMAs (including xbar transposes) execute serially in emission order
(Tile serializes DMATranspose vs DMACopy transitions), so the weight
load is split by h-slice and only slice 0 gates the first matmuls; the
rest stream in behind batch row 0's transposes.
Softmax over S (tiny, [32, 2048]) runs on host in fp32.
"""

import sys

try:
    import concourse.bass as bass  # noqa: F401
except ImportError:
    sys.path.insert(0, "/opt/trn_rl_repo")

import numpy as np
import ml_dtypes

import concourse.bacc as bacc
import concourse.mybir as mybir
import concourse.tile as tile
from concourse.bass_utils import run_bass_kernel_spmd

HID = 1024
BATCH = 32
SRC_LEN = 2048

N_CORES = 8
B_LOC = BATCH // N_CORES      # 4
E = 2 * HID                   # 2048
SG = 512                      # matmul moving free dim (s per group)
N_SG = SRC_LEN // SG          # 4
N_EC = E // 128               # 16 e-chunks
N_HC = HID // 128             # 8 h-chunks

f32 = mybir.dt.float32
bf16 = mybir.dt.bfloat16

_NC_CACHE = {}


def _build():
    nc = bacc.Bacc(
        "TRN2", target_bir_lowering=False, debug=False, num_devices=N_CORES
    )
    enc = nc.declare_dram_parameter("enc", [B_LOC, SRC_LEN, E], bf16, isOutput=False)
    wT = nc.declare_dram_parameter("wT", [N_HC, 128, N_EC * 128], bf16, isOutput=False)
    cb = nc.declare_dram_parameter("cb", [128, B_LOC * N_HC], f32, isOutput=False)
    vT = nc.declare_dram_parameter("vT", [128, N_HC], bf16, isOutput=False)
    att = nc.declare_dram_parameter("att", [N_SG, B_LOC, SG], f32, isOutput=True)

    with tile.TileContext(nc) as tc:
        with (
            tc.tile_pool(name="const", bufs=1) as const_pool,
            tc.tile_pool(name="encT", bufs=2 * N_SG) as encT_pool,
            tc.tile_pool(name="energy", bufs=8) as en_pool,
            tc.tile_pool(name="attsb", bufs=1) as att_pool,
            tc.tile_pool(name="psum", bufs=7, space="PSUM") as psum_pool,
            tc.tile_pool(name="attps", bufs=1, space="PSUM") as attps_pool,
        ):
            # weights stored h-slice-major: w_sb[:, hc, c, :] is the [128,128]
            # stationary for (e-chunk c, h-chunk hc); the host pre-lays-out wT
            # as [hc][p][c*128+h'] so each h-slice is one fully-contiguous DMA
            w_sb = const_pool.tile([128, N_HC, N_EC, 128], bf16)
            cb_sb = const_pool.tile([128, B_LOC * N_HC], f32)
            vT_sb = const_pool.tile([128, N_HC], bf16)
            att_all = att_pool.tile([128, B_LOC * SG], f32)

            def load_w_slice(hc):
                nc.sync.dma_start(
                    w_sb[:, hc].rearrange("p c h -> p (c h)"),
                    wT[hc],
                )

            # h-slice 0 of the weights + the small consts: everything the
            # first matmul burst needs; slices 1-2 follow the first transpose
            load_w_slice(0)
            nc.sync.dma_start(cb_sb[:], cb[:])
            nc.sync.dma_start(vT_sb[:], vT[:])

            # warmup tanh: pulls the ACT LUT-table-load dependency off the
            # first real tanh so no instruction needs multiple sync waits
            warm = const_pool.tile([128, 1], f32)
            nc.scalar.activation(
                warm[:], cb_sb[:, 0:1], mybir.ActivationFunctionType.Tanh
            )

            pending = []      # (att_ps, energy, hc, sg)
            done_banks = []   # (att_ps, b) fully accumulated

            def emit_vdots():
                while pending:
                    p_att, p_en, p_hc, p_sg = pending.pop(0)
                    nc.tensor.matmul(
                        p_att[32 * p_sg:32 * p_sg + 1, :],
                        lhsT=vT_sb[:, p_hc:p_hc + 1],
                        rhs=p_en[:],
                        start=(p_hc == 0),
                        stop=(p_hc == N_HC - 1),
                        tile_position=(0, 32 * p_sg),
                    )

            def drain_banks():
                while done_banks:
                    d_ps, d_b = done_banks.pop(0)
                    nc.vector.tensor_copy(
                        att_all[:, d_b * SG:(d_b + 1) * SG], d_ps[:]
                    )

            HALF = N_EC // 2

            def transpose_group(b, sg):
                encT = encT_pool.tile(
                    [128, N_EC, SG], bf16, tag="encT", name=f"encT_{b}_{sg}"
                )
                nc.sync.dma_start(
                    out=encT[:],
                    in_=enc[b, sg * SG:(sg + 1) * SG, :],
                    transpose=True,
                )
                return encT

            def transpose_half(b, sg, part):
                # separate tile per half: tile-granular dependency tracking
                # means a shared tile would make consumers wait for both DMAs
                encT_h = encT_pool.tile(
                    [128, HALF, SG], bf16, tag=f"encTh{part}",
                    name=f"encTh_{b}_{sg}_{part}",
                )
                nc.sync.dma_start(
                    out=encT_h[:],
                    in_=enc[
                        b,
                        sg * SG:(sg + 1) * SG,
                        part * HALF * 128:(part + 1) * HALF * 128,
                    ],
                    transpose=True,
                )
                return encT_h

            def enc_rhs(entry, c):
                if isinstance(entry, tuple):
                    return entry[c // HALF][:, c % HALF, :]
                return entry[:, c, :]

            def tanh_energy(pss, b, hc, att_ps):
                for sg in range(N_SG):
                    energy = en_pool.tile(
                        [128, SG], bf16, tag="en", name=f"en_{b}_{hc}_{sg}"
                    )
                    nc.scalar.activation(
                        energy[:], pss[sg][:],
                        mybir.ActivationFunctionType.Tanh,
                        bias=cb_sb[:, b * N_HC + hc:b * N_HC + hc + 1],
                        scale=1.0,
                    )
                    pending.append((att_ps, energy, hc, sg))

            N_WARM_HC = 3  # h-chunks runnable per tile during the startup ramp

            for b in range(B_LOC):
                if b == 0:
                    # serial DMA chain ordered so the PE is never starved:
                    # sg0's tile, then weight slices 1-2 (one burst each of
                    # ramp work per tile), remaining tiles, remaining slices
                    encTs = [transpose_group(0, 0)]
                    load_w_slice(1)
                    load_w_slice(2)
                    encTs += [
                        transpose_group(0, sg) for sg in range(1, N_SG)
                    ]
                    for hc in range(N_WARM_HC, N_HC):
                        load_w_slice(hc)
                else:
                    encTs = [
                        transpose_group(b, sg) for sg in range(N_SG)
                    ]
                att_ps = attps_pool.tile(
                    [128, SG], f32, tag="attps", name=f"attps_{b}"
                )
                hc_start = 0
                if b == 0:
                    # startup ramp: consume each s-group tile as its transpose
                    # lands, running h-chunks 0-2 per tile (their weight
                    # slices are the only ones loaded yet)
                    hc_start = N_WARM_HC
                    for sg in range(N_SG):
                        for hc in range(N_WARM_HC):
                            ps = psum_pool.tile(
                                [128, SG], f32, tag="ps", name=f"ps0_{hc}_{sg}"
                            )
                            for c in range(N_EC):
                                nc.tensor.matmul(
                                    ps[:],
                                    lhsT=w_sb[:, hc, c, :],
                                    rhs=encTs[sg][:, c, :],
                                    start=(c == 0),
                                    stop=(c == N_EC - 1),
                                )
                                if hc == 0 and c == 1:
                                    emit_vdots()
                            energy = en_pool.tile(
                                [128, SG], bf16, tag="en", name=f"en0_{hc}_{sg}"
                            )
                            nc.scalar.activation(
                                energy[:], ps[:],
                                mybir.ActivationFunctionType.Tanh,
                                bias=cb_sb[:, 0 * N_HC + hc:0 * N_HC + hc + 1],
                                scale=1.0,
                            )
                            pending.append((att_ps, energy, hc, sg))
                for hc in range(hc_start, N_HC):
                    last_burst = b == B_LOC - 1 and hc == N_HC - 1
                    if last_burst:
                        # tail: per-s-group bursts with eagerly interleaved
                        # tanh + v-dot so only sg3's epilogue trails the MMs
                        for sg in range(N_SG):
                            ps = psum_pool.tile(
                                [128, SG], f32, tag="ps", name=f"ps_t_{sg}"
                            )
                            for c in range(N_EC):
                                nc.tensor.matmul(
                                    ps[:],
                                    lhsT=w_sb[:, hc, c, :],
                                    rhs=encTs[sg][:, c, :],
                                    start=(c == 0),
                                    stop=(c == N_EC - 1),
                                )
                                if c == 1:
                                    emit_vdots()
                            energy = en_pool.tile(
                                [128, SG], bf16, tag="en", name=f"en_t_{sg}"
                            )
                            nc.scalar.activation(
                                energy[:], ps[:],
                                mybir.ActivationFunctionType.Tanh,
                                bias=cb_sb[:, b * N_HC + hc:b * N_HC + hc + 1],
                                scale=1.0,
                            )
                            pending.append((att_ps, energy, hc, sg))
                        continue
                    pss = [
                        psum_pool.tile([128, SG], f32, tag="ps", name=f"ps_{b}_{hc}_{i}")
                        for i in range(N_SG)
                    ]
                    for c in range(N_EC):
                        for sg in range(N_SG):
                            nc.tensor.matmul(
                                pss[sg][:],
                                lhsT=w_sb[:, hc, c, :],
                                rhs=encTs[sg][:, c, :],
                                start=(c == 0),
                                stop=(c == N_EC - 1),
                            )
                        if c == 1:
                            emit_vdots()
                            drain_banks()
                    tanh_energy(pss, b, hc, att_ps)
                done_banks.append((att_ps, b))
            emit_vdots()
            drain_banks()
            nc.sync.dma_start(
                att[:],
                att_all.rearrange("(q t) (b s) -> q t b s", t=32, b=B_LOC)[:, 0],
            )
    nc.compile()
    return nc


def _get_nc():
    if "nc" not in _NC_CACHE:
        _NC_CACHE["nc"] = _build()
    return _NC_CACHE["nc"]


def kernel(hidden, encoder_outputs, attn_w, attn_b, v_w, _trace=False):
    hidden = np.asarray(hidden, dtype=np.float32)
    encoder_outputs = np.asarray(encoder_outputs, dtype=np.float32)
    attn_w = np.asarray(attn_w, dtype=np.float32)
    attn_b = np.asarray(attn_b, dtype=np.float32)
    v_w = np.asarray(v_w, dtype=np.float32)

    # host-side prologue (tiny): h_proj + bias
    c_b = hidden @ attn_w[:, :HID].T + attn_b          # [B, H] fp32
    w_e = attn_w[:, HID:]                                  # [H, E]
    wT_bf = np.ascontiguousarray(
        w_e.reshape(N_HC, 128, N_EC, 128).transpose(0, 3, 2, 1)
        .reshape(N_HC, 128, N_EC * 128)
    ).astype(ml_dtypes.bfloat16)
    vT_dev = np.ascontiguousarray(
        v_w.reshape(N_HC, 128).T
    ).astype(ml_dtypes.bfloat16)

    nc = _get_nc()
    in_maps = []
    for core in range(N_CORES):
        b0 = core * B_LOC
        enc_bf = np.ascontiguousarray(
            encoder_outputs[:, b0:b0 + B_LOC, :].transpose(1, 0, 2)
        ).astype(ml_dtypes.bfloat16)
        cb_dev = np.ascontiguousarray(
            c_b[b0:b0 + B_LOC]
            .reshape(B_LOC, N_HC, 128)
            .transpose(2, 0, 1)
            .reshape(128, B_LOC * N_HC)
        )
        in_maps.append({"enc": enc_bf, "wT": wT_bf, "cb": cb_dev, "vT": vT_dev})

    res = run_bass_kernel_spmd(
        nc, in_maps, core_ids=list(range(N_CORES)), trace=_trace
    )
    if _trace:
        _NC_CACHE["last_result"] = res

    att = np.concatenate(
        [
            res.results[c]["att"].transpose(1, 0, 2).reshape(B_LOC, SRC_LEN)
            for c in range(N_CORES)
        ],
        axis=0,
    )  # [B, S] logits, fp32

    # host softmax over S
    m = att.max(axis=1, keepdims=True)
    e = np.exp(att - m)
    out = e / e.sum(axis=1, keepdims=True)
    return out.astype(np.float32)


# revision 17
# speedup vs baseline: 1.0202x; 1.0049x over previous
"""Trainium2 Bass kernel for nn_Attention (Bahdanau-style additive attention).

Reference computation:
    enc = encoder_outputs.transpose(1, 0, 2)            # [B, S, 2H]
    e_proj = enc @ w_e.T                                # [B, S, H]
    energy = tanh(h_proj[:, None, :] + e_proj + b)      # [B, S, H]
    att = energy @ v_w                                  # [B, S]
    out = softmax(att, axis=1)

Sharding: data-parallel over batch, 4 batch rows per core on 8 cores.
Per-core pipeline (all heavy compute in bf16 on the PE):
  - the encoder slice is DMA-transposed (xbar) from DRAM bf16 [S, 2H]
    into SBUF [128, 16, 512] tiles so the contraction dim (e) lands on
    partitions; one tile per 512 source positions
  - main matmul: psum[s-tile(128), h(512)] = sum_e enc^T chunk (the PE
    stationary, reused for both h-groups) @ w_e^T chunk; 16 e-chunks
    accumulate per bank
  - epilogue on the otherwise-idle Vector/Scalar engines:
    DVE adds the host-precomputed broadcast bias c_b = h_proj + attn_b,
    ACT applies tanh, DVE multiplies by v_w and reduces over h (free
    axis) straight into the attention logit column
  - batch row 0 ramps h-slice segments as its transposes land so the PE
    starts ~16us in; subsequent rows prefetch transposes inside the
    previous row's compute
h_proj ([32,1024] @ [1024,1024]) and the final softmax over [32, 2048]
are tiny and run on the host in fp32.
"""

import sys

try:
    import concourse.bass as bass  # noqa: F401
except ImportError:
    sys.path.insert(0, "/opt/trn_rl_repo")

import numpy as np
import ml_dtypes

import concourse.bacc as bacc
import concourse.mybir as mybir
import concourse.tile as tile
from concourse.bass_utils import run_bass_kernel_spmd

HID = 1024
BATCH = 32
SRC_LEN = 2048

N_CORES = 8
B_LOC = BATCH // N_CORES      # 4
E = 2 * HID                   # 2048
SG = 512                      # s per encoder transpose tile
N_SG = SRC_LEN // SG          # 4
N_EC = E // 128               # 16 e-chunks
N_HC = HID // 128             # 8 h-slices
N_ST = SRC_LEN // 128         # 16 s-tiles per batch row
HG = 512                      # h per psum bank
N_HG = HID // HG              # 2 h-groups

f32 = mybir.dt.float32
bf16 = mybir.dt.bfloat16

_NC_CACHE = {}


def _build():
    nc = bacc.Bacc(
        "TRN2", target_bir_lowering=False, debug=False, num_devices=N_CORES
    )
    enc = nc.declare_dram_parameter("enc", [B_LOC, SRC_LEN, E], bf16, isOutput=False)
    wT = nc.declare_dram_parameter("wT", [N_HC, 128, N_EC * 128], bf16, isOutput=False)
    cbb = nc.declare_dram_parameter("cbb", [B_LOC, 128, HID], f32, isOutput=False)
    vb = nc.declare_dram_parameter("vb", [128, HID], bf16, isOutput=False)
    # [b, p, st]: logit(b, st*128 + p)
    att = nc.declare_dram_parameter("att", [B_LOC, 128, N_ST], f32, isOutput=True)

    with tile.TileContext(nc) as tc:
        with (
            tc.tile_pool(name="const", bufs=1) as const_pool,
            tc.tile_pool(name="cbbp", bufs=2) as cbb_pool,
            tc.tile_pool(name="encT", bufs=6) as encT_pool,
            tc.tile_pool(name="tanhE", bufs=18) as te_pool,
            tc.tile_pool(name="scratch", bufs=3) as sc_pool,
            tc.tile_pool(name="attsb", bufs=1) as att_pool,
            tc.tile_pool(name="psum", bufs=5, space="PSUM") as psum_pool,
            tc.tile_pool(name="psumr", bufs=3, space="PSUM") as psumr_pool,
        ):
            w_sb = const_pool.tile([128, N_HC, N_EC, 128], bf16)
            vb_sb = const_pool.tile([128, HID], bf16)
            att_sb = att_pool.tile([128, B_LOC * N_ST], f32)

            def load_w_slice(hs):
                nc.sync.dma_start(
                    w_sb[:, hs].rearrange("p c h -> p (c h)"), wT[hs]
                )

            cbb_sbs = [None] * B_LOC

            def load_cbb(b):
                t = cbb_pool.tile([128, HID], f32, tag="cbb", name=f"cbb_{b}")
                nc.sync.dma_start(t[:], cbb[b])
                cbb_sbs[b] = t

            def transpose_group(b, sg):
                encT = encT_pool.tile(
                    [128, N_EC, SG], bf16, tag="encT", name=f"encT_{b}_{sg}"
                )
                nc.sync.dma_start(
                    out=encT[:],
                    in_=enc[b, sg * SG:(sg + 1) * SG, :],
                    transpose=True,
                )
                return encT

            # startup DMA order on the serial chain: just enough weight
            # for the first ramp segment before the first transpose
            load_w_slice(0)
            load_w_slice(1)

            # warmup tanh for the ACT LUT-table dependency
            warm = const_pool.tile([128, 1], f32)
            nc.scalar.activation(
                warm[:], w_sb[:, 0, 0, 0:1], mybir.ActivationFunctionType.Tanh
            )

            def lhs_enc(encT, st, c):
                j = st % N_SG
                return encT[:, c, j * 128:(j + 1) * 128]

            def epilogue_half(b, st, ps, hg, tanhE):
                # energy = tanh(psum + c_b), half h-group at a time
                pre = sc_pool.tile(
                    [128, HG], bf16, tag="pre", name=f"pre_{b}_{st}_{hg}"
                )
                nc.vector.tensor_add(
                    out=pre[:],
                    in0=ps[:],
                    in1=cbb_sbs[b][:, hg * HG:(hg + 1) * HG],
                )
                nc.scalar.activation(
                    tanhE[:, hg * HG:(hg + 1) * HG], pre[:],
                    mybir.ActivationFunctionType.Tanh,
                )

            def vdot(b, st, tanhE):
                # energy * v then reduce over h (free axis), both on DVE
                outj = sc_pool.tile(
                    [128, HID], bf16, tag="ttr", name=f"ttr_{b}_{st}"
                )
                nc.vector.tensor_mul(out=outj[:], in0=tanhE[:], in1=vb_sb[:])
                nc.vector.tensor_reduce(
                    att_sb[:, b * N_ST + st:b * N_ST + st + 1],
                    outj[:],
                    mybir.AxisListType.X,
                    mybir.AluOpType.add,
                )

            # ---- batch row 0: ramp as transposes land ----
            # pass 1: h-group 0 per s-tile in two h-slice-pair segments;
            # each segment is one accumulation group on the bank and is
            # drained before the next segment reopens the zero region
            encTs = []
            tanhEs = {}
            for sg in range(N_SG):
                encTs.append(transpose_group(0, sg))
                if sg == 0:
                    load_cbb(0)
                    load_w_slice(2)
                    load_w_slice(3)
                elif sg == 1:
                    for hs in range(4, 6):
                        load_w_slice(hs)
                elif sg == 2:
                    for hs in range(6, N_HC):
                        load_w_slice(hs)
            nc.sync.dma_start(vb_sb[:], vb[:])
            for sg in range(N_SG):
                for st in range(sg * N_SG, (sg + 1) * N_SG):
                    tanhEs[st] = te_pool.tile(
                        [128, HID], bf16, tag="te", name=f"te0_{st}"
                    )
                for seg in range(2):
                    for st in range(sg * N_SG, (sg + 1) * N_SG):
                        ps = psumr_pool.tile(
                            [128, 256], f32, tag="psr", name=f"psr_{st}_{seg}"
                        )
                        for c in range(N_EC):
                            for hh in range(2):
                                hs = seg * 2 + hh
                                nc.tensor.matmul(
                                    ps[:, hh * 128:(hh + 1) * 128],
                                    lhsT=lhs_enc(encTs[sg], st, c),
                                    rhs=w_sb[:, hs, c, :],
                                    start=(c == 0 and hh == 0),
                                    stop=(c == N_EC - 1 and hh == 1),
                                )
                        pre = sc_pool.tile(
                            [128, 256], bf16, tag="prer", name=f"prer_{st}_{seg}"
                        )
                        nc.vector.tensor_add(
                            out=pre[:],
                            in0=ps[:],
                            in1=cbb_sbs[0][:, seg * 256:(seg + 1) * 256],
                        )
                        nc.scalar.activation(
                            tanhEs[st][:, seg * 256:(seg + 1) * 256], pre[:],
                            mybir.ActivationFunctionType.Tanh,
                        )
            # pass 2: h-group 1 + v-dot per s-tile; prefetch b1's tiles
            encTs_next = []
            for st in range(N_ST):
                sg = st // N_SG
                ps1 = psum_pool.tile([128, HG], f32, tag="ps", name=f"ps1_{st}")
                for c in range(N_EC):
                    nc.tensor.matmul(
                        ps1[:],
                        lhsT=lhs_enc(encTs[sg], st, c),
                        rhs=w_sb[:, 4:8, c, :],
                        start=(c == 0),
                        stop=(c == N_EC - 1),
                    )
                if st == 0:
                    encTs_next.append(transpose_group(1, 0))
                    load_cbb(1)
                elif st in (2, 5, 9):
                    encTs_next.append(transpose_group(1, len(encTs_next)))
                epilogue_half(0, st, ps1, 1, tanhEs[st])
                vdot(0, st, tanhEs[st])
            nc.sync.dma_start(att[0], att_sb[:, 0:N_ST])

            # ---- batch rows 1..3: steady state ----
            for b in range(1, B_LOC):
                encTs = encTs_next
                encTs_next = []
                for st in range(N_ST):
                    sg = st // N_SG
                    if b < B_LOC - 1:
                        if st == 1:
                            encTs_next.append(transpose_group(b + 1, 0))
                            load_cbb(b + 1)
                        elif st in (3, 6, 10):
                            encTs_next.append(transpose_group(b + 1, len(encTs_next)))
                    ps = [
                        psum_pool.tile(
                            [128, HG], f32, tag="ps", name=f"ps_{b}_{st}_{g}"
                        )
                        for g in range(N_HG)
                    ]
                    for c in range(N_EC):
                        for hg in range(N_HG):
                            nc.tensor.matmul(
                                ps[hg][:],
                                lhsT=lhs_enc(encTs[sg], st, c),
                                rhs=w_sb[:, hg * 4:(hg + 1) * 4, c, :],
                                start=(c == 0),
                                stop=(c == N_EC - 1),
                            )
                    tanhE = te_pool.tile(
                        [128, HID], bf16, tag="te", name=f"te_{b}_{st}"
                    )
                    for hg in range(N_HG):
                        epilogue_half(b, st, ps[hg], hg, tanhE)
                    vdot(b, st, tanhE)
                nc.sync.dma_start(att[b], att_sb[:, b * N_ST:(b + 1) * N_ST])
    nc.compile()
    return nc


def _get_nc():
    if "nc" not in _NC_CACHE:
        _NC_CACHE["nc"] = _build()
    return _NC_CACHE["nc"]


def kernel(hidden, encoder_outputs, attn_w, attn_b, v_w, _trace=False):
    hidden = np.asarray(hidden, dtype=np.float32)
    encoder_outputs = np.asarray(encoder_outputs, dtype=np.float32)
    attn_w = np.asarray(attn_w, dtype=np.float32)
    attn_b = np.asarray(attn_b, dtype=np.float32)
    v_w = np.asarray(v_w, dtype=np.float32)

    c_b = hidden @ attn_w[:, :HID].T + attn_b          # [B, H] fp32
    w_e = attn_w[:, HID:]                              # [H, E]
    wT_bf = np.ascontiguousarray(
        w_e.reshape(N_HC, 128, N_EC, 128).transpose(0, 3, 2, 1)
        .reshape(N_HC, 128, N_EC * 128)
    ).astype(ml_dtypes.bfloat16)
    vb_dev = np.ascontiguousarray(
        np.broadcast_to(v_w[None, :], (128, HID))
    ).astype(ml_dtypes.bfloat16)

    nc = _get_nc()
    in_maps = []
    for core in range(N_CORES):
        b0 = core * B_LOC
        enc_bf = np.ascontiguousarray(
            encoder_outputs[:, b0:b0 + B_LOC, :].transpose(1, 0, 2)
        ).astype(ml_dtypes.bfloat16)
        cbb_dev = np.ascontiguousarray(
            np.broadcast_to(c_b[b0:b0 + B_LOC, None, :], (B_LOC, 128, HID))
        ).astype(np.float32)
        in_maps.append(
            {"enc": enc_bf, "wT": wT_bf, "cbb": cbb_dev, "vb": vb_dev}
        )

    res = run_bass_kernel_spmd(
        nc, in_maps, core_ids=list(range(N_CORES)), trace=_trace
    )
    if _trace:
        _NC_CACHE["last_result"] = res

    att = np.concatenate(
        [
            res.results[c]["att"].transpose(0, 2, 1).reshape(B_LOC, SRC_LEN)
            for c in range(N_CORES)
        ],
        axis=0,
    )  # [B, S] logits

    m = att.max(axis=1, keepdims=True)
    e = np.exp(att - m)
    out = e / e.sum(axis=1, keepdims=True)
    return out.astype(np.float32)
